# revision 1
# baseline (speedup 1.0000x reference)
"""AttentionBlock (GroupNorm + 4-head self-attention + proj + residual) on 8 TRN2 cores.

Sharding: core = 2*b + hh  (b = batch 0..3, hh = head-half 0..1).
Each core handles one batch image and 2 of the 4 heads (tensor-parallel over
heads for qkv/attention/proj).  GroupNorm (cheap) is recomputed on both cores
of a batch.  Each core emits a partial proj output [256, 4096]; the host sums
the two head-half partials, adds proj bias and the residual x.
"""

import sys

sys.path.insert(0, "/opt/trn_rl_repo")

import numpy as np  # noqa: E402

import concourse.bacc as bacc  # noqa: E402
import concourse.tile as tile  # noqa: E402
from concourse import mybir  # noqa: E402
from concourse.bass_utils import run_bass_kernel_spmd  # noqa: E402
from concourse.masks import make_identity  # noqa: E402

F32 = mybir.dt.float32
F32R = mybir.dt.float32r
BF16 = mybir.dt.bfloat16
AF = mybir.ActivationFunctionType
ALU = mybir.AluOpType

# Problem constants (hardcoded per contract)
B, C, H, W = 4, 256, 64, 64
N = H * W          # 4096 pixels
NH, HD = 4, 64     # heads, head dim
GROUPS = 8
EPS = 1e-5
SCALE = HD ** -0.5  # 0.125

NCHUNK = 512            # pixel chunk (matmul moving dim)
NCH = N // NCHUNK       # 8
MCH = N // 128          # 32 m-chunks of 128 pixels
EXPG = 2                # m-chunks exp'd per ACT instruction


def build_bass():
    nc = bacc.Bacc("TRN2", target_bir_lowering=False, debug=False)

    # ---- DRAM I/O (per-core shards fed via in_maps) ----
    xd = nc.dram_tensor("x", [C, N], F32, kind="ExternalInput")
    wqkvT_d = nc.dram_tensor("wqkvT", [C, 384], F32R, kind="ExternalInput")
    qkvb_d = nc.dram_tensor("qkvb", [3, 128, 1], F32, kind="ExternalInput")
    pwT_d = nc.dram_tensor("pwT", [128, C], F32R, kind="ExternalInput")
    nw_d = nc.dram_tensor("nw", [2, 128, 1], F32, kind="ExternalInput")
    nb_d = nc.dram_tensor("nb", [2, 128, 1], F32, kind="ExternalInput")
    indf_d = nc.dram_tensor("indf", [2, 128, 8], F32, kind="ExternalInput")
    indb_d = nc.dram_tensor("indb", [2, 8, 128], F32, kind="ExternalInput")
    out_d = nc.dram_tensor("out_part", [C, N], F32, kind="ExternalOutput")

    with tile.TileContext(nc) as tc:
        with (
            tc.tile_pool(name="persist", bufs=1) as pp,
            tc.tile_pool(name="tmp", bufs=4) as tp,
            tc.tile_pool(name="small", bufs=4) as sp,
            tc.tile_pool(name="apool", bufs=6) as ap_pool,
            tc.tile_pool(name="ps_s", bufs=2, space="PSUM") as ps_s,
            tc.tile_pool(name="ps_o", bufs=2, space="PSUM") as ps_o,
        ):
            # ================= Phase 0: loads & constants =================
            x_t = []
            for i in range(2):
                xt = pp.tile([128, N], F32, tag=f"x{i}", name=f"x{i}")
                for c4 in range(4):
                    nc.sync.dma_start(
                        out=xt[:, 1024 * c4:1024 * (c4 + 1)],
                        in_=xd[128 * i:128 * (i + 1), 1024 * c4:1024 * (c4 + 1)])
                x_t.append(xt)

            wqkvT_t = []
            for i in range(2):
                wt = pp.tile([128, 384], F32R, tag=f"wqkv{i}", name=f"wqkv{i}")
                nc.sync.dma_start(out=wt, in_=wqkvT_d[128 * i:128 * (i + 1), :])
                wqkvT_t.append(wt)

            qkvb_t = []
            for j in range(3):
                bt = sp.tile([128, 1], F32, tag=f"qkvb{j}", name=f"qkvb{j}")
                nc.sync.dma_start(out=bt, in_=qkvb_d[j])
                qkvb_t.append(bt)

            pwT_t = []
            for i in range(2):
                pt = pp.tile([128, 128], F32R, tag=f"pw{i}", name=f"pw{i}")
                nc.sync.dma_start(out=pt, in_=pwT_d[:, 128 * i:128 * (i + 1)])
                pwT_t.append(pt)

            nw_t, nb_t, indf_t, indb_t = [], [], [], []
            for i in range(2):
                t1 = sp.tile([128, 1], F32, tag=f"nw{i}", name=f"nw{i}")
                nc.sync.dma_start(out=t1, in_=nw_d[i])
                nw_t.append(t1)
                t2 = sp.tile([128, 1], F32, tag=f"nb{i}", name=f"nb{i}")
                nc.sync.dma_start(out=t2, in_=nb_d[i])
                nb_t.append(t2)
                t3 = sp.tile([128, 8], F32, tag=f"indf{i}", name=f"indf{i}")
                nc.sync.dma_start(out=t3, in_=indf_d[i])
                indf_t.append(t3)
                t4 = sp.tile([8, 128], F32, tag=f"indb{i}", name=f"indb{i}")
                nc.sync.dma_start(out=t4, in_=indb_d[i])
                indb_t.append(t4)

            ident = pp.tile([128, 128], BF16, tag="ident", name="ident")
            make_identity(nc, ident)

            eps8 = sp.tile([8, 1], F32, tag="eps8", name="eps8")
            nc.vector.memset(eps8, EPS)

            # ================= Phase 1: GroupNorm =================
            SDIM = nc.vector.BN_STATS_DIM   # 6
            ADIM = nc.vector.BN_AGGR_DIM    # 2
            NSUB = N // nc.vector.BN_STATS_FMAX if N > nc.vector.BN_STATS_FMAX else 1
            SUBLEN = N // NSUB

            m1e2 = []
            for i in range(2):
                st = tp.tile([128, NSUB, SDIM], F32, tag="bnst", name=f"bnst{i}")
                for s in range(NSUB):
                    nc.vector.bn_stats(
                        out=st[:, s, :],
                        in_=x_t[i][:, SUBLEN * s:SUBLEN * (s + 1)],
                    )
                mv = tp.tile([128, ADIM], F32, tag="bnmv", name=f"bnmv{i}")
                nc.vector.bn_aggr(out=mv, in_=st)
                # build [mean, E[x^2]] = [mean, var + mean^2]
                me = sp.tile([128, 2], F32, tag=f"m1e2_{i}", name=f"m1e2_{i}")
                msq = tp.tile([128, 1], F32, tag="msq", name=f"msq{i}")
                nc.vector.tensor_mul(out=msq, in0=mv[:, 0:1], in1=mv[:, 0:1])
                nc.vector.tensor_copy(out=me[:, 0:1], in_=mv[:, 0:1])
                nc.vector.tensor_add(out=me[:, 1:2], in0=mv[:, 1:2], in1=msq)
                m1e2.append(me)

            # group sums: psum[8, 2] = sum_c ind[c, g] * [mean_c, e2_c]
            psg = ps_s.tile([8, 2], F32, tag="s", name="psg")
            nc.tensor.matmul(psg, lhsT=indf_t[0], rhs=m1e2[0], start=True, stop=False)
            nc.tensor.matmul(psg, lhsT=indf_t[1], rhs=m1e2[1], start=False, stop=True)

            sg = sp.tile([8, 2], F32, tag="sg", name="sg")
            nc.scalar.mul(out=sg, in_=psg, mul=1.0 / 32.0)  # [mean_g, e2_g]
            vg = sp.tile([8, 1], F32, tag="vg", name="vg")
            nc.vector.tensor_mul(out=vg, in0=sg[:, 0:1], in1=sg[:, 0:1])
            nc.vector.tensor_sub(out=vg, in0=sg[:, 1:2], in1=vg)  # var_g
            nc.scalar.activation(out=vg, in_=vg, func=AF.Sqrt, bias=eps8)
            nc.vector.reciprocal(out=sg[:, 1:2], in_=vg)          # rstd_g into col 1

            h_t = []
            for i in range(2):
                psc = ps_s.tile([128, 2], F32, tag="s", name=f"psc{i}")
                nc.tensor.matmul(psc, lhsT=indb_t[i], rhs=sg, start=True, stop=True)
                sc = sp.tile([128, 1], F32, tag=f"sc{i}", name=f"sc{i}")
                off = sp.tile([128, 1], F32, tag=f"off{i}", name=f"off{i}")
                nc.vector.tensor_mul(out=sc, in0=psc[:, 1:2], in1=nw_t[i])
                nc.vector.tensor_mul(out=off, in0=psc[:, 0:1], in1=sc)
                nc.vector.tensor_sub(out=off, in0=nb_t[i], in1=off)
                ht = pp.tile([128, N], F32R, tag=f"h{i}", name=f"h{i}")
                for c4 in range(4):
                    csl = slice(1024 * c4, 1024 * (c4 + 1))
                    nc.vector.tensor_scalar(
                        out=ht[:, csl], in0=x_t[i][:, csl], scalar1=sc, scalar2=off,
                        op0=ALU.mult, op1=ALU.add,
                    )
                h_t.append(ht)

            # ================= Phase 2: qkv (o-layout) =================
            qT = pp.tile([128, N], BF16, tag="qT", name="qT")
            kT = pp.tile([128, N], BF16, tag="kT", name="kT")
            vT = pp.tile([128, N], BF16, tag="vT", name="vT")
            # v in [pixel, d] layout: v_all[:, j, :] = [vA | ones | vB | ones];
            # attnv lhsT slices [0:65]/[65:130] put the softmax sums at row 64.
            v_all = pp.tile([128, MCH, 130], BF16, tag="v_all", name="v_all")
            nc.gpsimd.memset(v_all[:, :, 64:65], 1.0)
            nc.gpsimd.memset(v_all[:, :, 129:130], 1.0)

            dests = [qT, kT, vT]
            # v first (transposing each chunk as it completes, in the idle
            # ps_o slots), then k, then q - so attention starts as early as
            # possible and late q chunks overlap the attention phase.
            for oi in (2, 1, 0):
                for n in range(NCH):
                    ps = ps_s.tile([128, NCHUNK], F32, tag="s", name=f"qkv{oi}_{n}")
                    for ci in range(2):
                        nc.tensor.matmul(
                            ps,
                            lhsT=wqkvT_t[ci][:, 128 * oi:128 * (oi + 1)],
                            rhs=h_t[ci][:, NCHUNK * n:NCHUNK * (n + 1)],
                            start=(ci == 0), stop=(ci == 1),
                        )
                    nc.vector.tensor_scalar(
                        out=dests[oi][:, NCHUNK * n:NCHUNK * (n + 1)],
                        in0=ps, scalar1=qkvb_t[oi], scalar2=None, op0=ALU.add,
                    )
                    if oi == 2:
                        for jj in range(4):
                            j = 4 * n + jj
                            pst = ps_o.tile([128, 128], BF16, tag=f"o{j % 2}",
                                            name=f"vtr{j}")
                            nc.tensor.transpose(pst, vT[:, 128 * j:128 * (j + 1)], ident)
                            nc.vector.tensor_copy(out=v_all[:, j, 0:64], in_=pst[:, 0:64])
                            nc.vector.tensor_copy(out=v_all[:, j, 65:129], in_=pst[:, 64:128])



            # ================= Phase 3: attention + proj =================
            # Software-pipelined: chunk n's normalization+proj tail is emitted
            # in the middle of chunk n+1's score/attnv group loop so the PE
            # never waits on the DVE/GPSIMD reciprocal chain.
            def emit_tail(po, n):
                nsl = slice(NCHUNK * n, NCHUNK * (n + 1))
                rbh = []
                for hh in range(2):
                    rr = tp.tile([1, NCHUNK], F32, tag=f"rr{hh}", name=f"rr{hh}_{n}")
                    nc.vector.tensor_copy(out=rr, in_=po[hh][64:65, :])
                    nc.vector.reciprocal_approx_fast(out=rr, in_=rr)
                    rb = tp.tile([64, NCHUNK], F32, tag=f"rb{hh}", name=f"rb{hh}_{n}")
                    nc.gpsimd.partition_broadcast(rb, rr, channels=64)
                    rbh.append(rb)
                onrm = tp.tile([128, NCHUNK], F32R, tag="onrm", name=f"onrm_{n}")
                nc.vector.tensor_mul(out=onrm[0:64, :], in0=po[0][0:64, :], in1=rbh[0])
                nc.vector.tensor_mul(out=onrm[64:128, :], in0=po[1][0:64, :], in1=rbh[1])
                for ci in range(2):
                    ppj = ps_s.tile([128, NCHUNK], F32, tag="s", name=f"proj{ci}_{n}")
                    nc.tensor.matmul(ppj, lhsT=pwT_t[ci], rhs=onrm, start=True, stop=True)
                    osb = tp.tile([128, NCHUNK], F32, tag="osb", name=f"osb{ci}_{n}")
                    nc.vector.tensor_copy(out=osb, in_=ppj)
                    nc.sync.dma_start(out=out_d[128 * ci:128 * (ci + 1), nsl], in_=osb)

            pending = None
            for n in range(NCH):
                nsl = slice(NCHUNK * n, NCHUNK * (n + 1))
                po = []
                for hh in range(2):
                    poh = ps_o.tile([65, NCHUNK], F32, tag=f"o{hh}", name=f"po{hh}_{n}")
                    dsl = slice(64 * hh, 64 * (hh + 1))
                    for g in range(MCH // EXPG):
                        pss = ps_s.tile([128, EXPG, NCHUNK], F32, tag="s", name=f"s{n}_{hh}_{g}")
                        for u in range(EXPG):
                            j = EXPG * g + u
                            nc.tensor.matmul(
                                pss[:, u, :],
                                lhsT=kT[dsl, 128 * j:128 * (j + 1)],
                                rhs=qT[dsl, nsl],
                                start=True, stop=True,
                            )
                        at = ap_pool.tile([128, EXPG, NCHUNK], BF16, tag="a", name=f"a{n}_{hh}_{g}")
                        nc.scalar.activation(out=at, in_=pss, func=AF.Exp, scale=SCALE)
                        for u in range(EXPG):
                            j = EXPG * g + u
                            lhs = v_all[:, j, 0:65] if hh == 0 else v_all[:, j, 65:130]
                            nc.tensor.matmul(
                                poh, lhsT=lhs, rhs=at[:, u, :],
                                start=(j == 0), stop=(j == MCH - 1),
                            )
                        if pending is not None and hh == 0 and g == 5:
                            emit_tail(*pending)
                            pending = None
                    po.append(poh)
                pending = (po, n)
            emit_tail(*pending)

    nc.compile()
    return nc


_NC_CACHE = None


def _get_nc():
    global _NC_CACHE
    if _NC_CACHE is None:
        _NC_CACHE = build_bass()
    return _NC_CACHE


def _make_in_maps(x, norm_w, norm_b, qkv_w, qkv_b, proj_w):
    # constant index helper tensors
    ch = np.arange(128)
    indf = np.zeros((2, 128, 8), np.float32)
    indb = np.zeros((2, 8, 128), np.float32)
    for i in range(2):
        g = (i * 128 + ch) // 32
        indf[i, ch, g] = 1.0
        indb[i, g, ch] = 1.0
    nw = norm_w.reshape(2, 128, 1).astype(np.float32)
    nb = norm_b.reshape(2, 128, 1).astype(np.float32)

    in_maps = []
    for core in range(8):
        b, hh = core // 2, core % 2
        sl = slice(128 * hh, 128 * (hh + 1))
        w_slice = np.concatenate(
            [qkv_w[sl], qkv_w[256 + 128 * hh:256 + 128 * (hh + 1)],
             qkv_w[512 + 128 * hh:512 + 128 * (hh + 1)]], axis=0,
        )  # [384, 256]
        wqkvT = np.ascontiguousarray(w_slice.T).astype(np.float32)  # [256, 384]
        qkvb = np.stack(
            [qkv_b[sl], qkv_b[256 + 128 * hh:256 + 128 * (hh + 1)],
             qkv_b[512 + 128 * hh:512 + 128 * (hh + 1)]], axis=0,
        ).reshape(3, 128, 1).astype(np.float32)
        pwT = np.ascontiguousarray(proj_w[:, sl].T).astype(np.float32)  # [128, 256]
        in_maps.append({
            "x": np.ascontiguousarray(x[b].reshape(C, N)).astype(np.float32),
            "wqkvT": wqkvT,
            "qkvb": qkvb,
            "pwT": pwT,
            "nw": nw,
            "nb": nb,
            "indf": indf,
            "indb": indb,
        })
    return in_maps


def kernel(x, norm_w, norm_b, qkv_w, qkv_b, proj_w, proj_b, _trace=False, _tmpdir=None):
    x = np.asarray(x, np.float32)
    norm_w = np.asarray(norm_w, np.float32)
    norm_b = np.asarray(norm_b, np.float32)
    qkv_w = np.asarray(qkv_w, np.float32)
    qkv_b = np.asarray(qkv_b, np.float32)
    proj_w = np.asarray(proj_w, np.float32)
    proj_b = np.asarray(proj_b, np.float32)

    nc = _get_nc()
    in_maps = _make_in_maps(x, norm_w, norm_b, qkv_w, qkv_b, proj_w)
    kw = {}
    if _trace:
        kw = dict(trace=True, tmpdir=_tmpdir)
    res = run_bass_kernel_spmd(nc, in_maps, list(range(8)), **kw)

    out = np.empty((B, C, H, W), np.float32)
    bias_res = proj_b[:, None].astype(np.float32)
    for b in range(B):
        acc = (res.results[2 * b]["out_part"] + res.results[2 * b + 1]["out_part"]
               + bias_res + x[b].reshape(C, N))
        out[b] = acc.reshape(C, H, W)
    if _trace:
        return out, res
    return out



# revision 4
# speedup vs baseline: 1.2799x; 1.2799x over previous
"""AttentionBlock (GroupNorm + 4-head self-attention + proj + residual) on 8 TRN2 cores.

Sharding: core = 2*b + hh  (b = batch 0..3, hh = head-half 0..1).
Each core handles one batch image and 2 of the 4 heads.

Key engine-level structure (vs the v1 baseline):
 - The two heads' score matmuls (K=64 contraction each) are issued adjacently so
   the PE runs them concurrently via row tiling (partitions 0:64 / 64:128).
 - The softmax exp (the ScalarE bottleneck: N^2 logits per head) is split across
   engines: head A -> ScalarE exact Exp LUT; head B -> VectorE "Schraudolph"
   approximate exp: int16(S*a+b) whose bits, reinterpreted as bf16, equal
   C * 2^(S*scale*log2e).  The constant factor C cancels in softmax.
 - k bias is dropped (constant per query -> cancels in softmax); q bias is fused
   into the ScalarE PSUM->SBUF copy; v bias is folded into the output on host.
 - The softmax division is moved to the host: the device ships raw per-head proj
   partials [256, N] plus the per-head denominator rows [2, N]; the host does
   out = sum_heads partial/denominator + const + residual.
"""

import sys

sys.path.insert(0, "/opt/trn_rl_repo")

import numpy as np  # noqa: E402

import concourse.bacc as bacc  # noqa: E402
import concourse.tile as tile  # noqa: E402
from concourse import mybir  # noqa: E402
from concourse.bass_utils import run_bass_kernel_spmd  # noqa: E402

F32 = mybir.dt.float32
BF16 = mybir.dt.bfloat16
I16 = mybir.dt.int16
AF = mybir.ActivationFunctionType
ALU = mybir.AluOpType

# Problem constants (hardcoded per contract)
B, C, H, W = 4, 256, 64, 64
N = H * W          # 4096 pixels
NH, HD = 4, 64     # heads, head dim
GROUPS = 8
EPS = 1e-5
SCALE = HD ** -0.5  # 0.125

NCHUNK = 512            # pixel chunk (matmul moving dim)
NCH = N // NCHUNK       # 8
MCH = N // 128          # 32 k-chunks of 128 pixels
SKEW = 3                # attnv lags scores by SKEW k-chunks

# Schraudolph exp-as-bf16-bits constants (DVE rounds to nearest; verified on HW)
LOG2E = 1.4426950408889634
A_C = SCALE * LOG2E * 128.0        # 23.0831...
B_C = 128.0 * (127.0 - 0.0430)     # 16250.496


def build_bass():
    nc = bacc.Bacc("TRN2", target_bir_lowering=False, debug=False)

    # ---- DRAM I/O (per-core shards fed via in_maps) ----
    xd = nc.dram_tensor("x", [C, N], F32, kind="ExternalInput")
    wqT_d = nc.dram_tensor("wqT", [2, 128, 128], BF16, kind="ExternalInput")
    wkT_d = nc.dram_tensor("wkT", [2, 128, 128], BF16, kind="ExternalInput")
    wvT_d = nc.dram_tensor("wvT", [2, 128, 130], BF16, kind="ExternalInput")
    qb_d = nc.dram_tensor("qb", [128, 1], F32, kind="ExternalInput")
    pwT_d = nc.dram_tensor("pwT", [128, 256], BF16, kind="ExternalInput")
    nw_d = nc.dram_tensor("nw", [2, 128, 1], F32, kind="ExternalInput")
    nb_d = nc.dram_tensor("nb", [2, 128, 1], F32, kind="ExternalInput")
    indf_d = nc.dram_tensor("indf", [2, 128, 8], F32, kind="ExternalInput")
    indb_d = nc.dram_tensor("indb", [2, 8, 128], F32, kind="ExternalInput")
    outA_d = nc.dram_tensor("outA", [C, N], F32, kind="ExternalOutput")
    outB_d = nc.dram_tensor("outB", [C, N], F32, kind="ExternalOutput")
    sums_d = nc.dram_tensor("sums", [2, N], F32, kind="ExternalOutput")

    with tile.TileContext(nc) as tc:
        with (
            tc.tile_pool(name="persist", bufs=1) as pp,
            tc.tile_pool(name="tmp", bufs=4) as tp,
            tc.tile_pool(name="small", bufs=4) as sp,
            tc.tile_pool(name="a0pool", bufs=6) as a0p,
            tc.tile_pool(name="a1pool", bufs=6) as a1p,
            tc.tile_pool(name="onpool", bufs=2) as onp,
            tc.tile_pool(name="osbpool", bufs=4) as obp,
            tc.tile_pool(name="ps_sc", bufs=4, space="PSUM") as ps_sc,
            tc.tile_pool(name="ps_po", bufs=1, space="PSUM") as ps_po,
            tc.tile_pool(name="ps_pj", bufs=2, space="PSUM") as ps_pj,
        ):
            # ================= Phase 0: loads & constants =================
            x_t = []
            for i in range(2):
                xt = pp.tile([128, N], F32, tag=f"x{i}", name=f"x{i}")
                for c4 in range(4):
                    nc.sync.dma_start(
                        out=xt[:, 1024 * c4:1024 * (c4 + 1)],
                        in_=xd[128 * i:128 * (i + 1), 1024 * c4:1024 * (c4 + 1)])
                x_t.append(xt)

            wqT_t, wkT_t, wvT_t = [], [], []
            for ci in range(2):
                t = pp.tile([128, 128], BF16, tag=f"wq{ci}", name=f"wq{ci}")
                nc.sync.dma_start(out=t, in_=wqT_d[ci])
                wqT_t.append(t)
                t = pp.tile([128, 128], BF16, tag=f"wk{ci}", name=f"wk{ci}")
                nc.sync.dma_start(out=t, in_=wkT_d[ci])
                wkT_t.append(t)
                t = pp.tile([128, 130], BF16, tag=f"wv{ci}", name=f"wv{ci}")
                nc.sync.dma_start(out=t, in_=wvT_d[ci])
                wvT_t.append(t)

            qb_t = sp.tile([128, 1], F32, tag="qb", name="qb")
            nc.sync.dma_start(out=qb_t, in_=qb_d[:, :])
            pwT_t = pp.tile([128, 256], BF16, tag="pw", name="pw")
            nc.sync.dma_start(out=pwT_t, in_=pwT_d[:, :])

            nw_t, nb_t, indf_t, indb_t = [], [], [], []
            for i in range(2):
                t1 = sp.tile([128, 1], F32, tag=f"nw{i}", name=f"nw{i}")
                nc.sync.dma_start(out=t1, in_=nw_d[i])
                nw_t.append(t1)
                t2 = sp.tile([128, 1], F32, tag=f"nb{i}", name=f"nb{i}")
                nc.sync.dma_start(out=t2, in_=nb_d[i])
                nb_t.append(t2)
                t3 = sp.tile([128, 8], F32, tag=f"indf{i}", name=f"indf{i}")
                nc.sync.dma_start(out=t3, in_=indf_d[i])
                indf_t.append(t3)
                t4 = sp.tile([8, 128], F32, tag=f"indb{i}", name=f"indb{i}")
                nc.sync.dma_start(out=t4, in_=indb_d[i])
                indb_t.append(t4)

            eps8 = sp.tile([8, 1], F32, tag="eps8", name="eps8")
            nc.vector.memset(eps8, EPS)

            # v_all[:, j, :] = [vA(64) | onesA(1) | vB(64) | onesB(1)]
            v_all = pp.tile([128, MCH, 130], BF16, tag="v_all", name="v_all")

            # ================= Phase 1: GroupNorm =================
            SDIM = nc.vector.BN_STATS_DIM   # 6
            ADIM = nc.vector.BN_AGGR_DIM    # 2
            NSUB = N // nc.vector.BN_STATS_FMAX if N > nc.vector.BN_STATS_FMAX else 1
            SUBLEN = N // NSUB

            m1e2 = []
            for i in range(2):
                st = tp.tile([128, NSUB, SDIM], F32, tag="bnst", name=f"bnst{i}")
                for s in range(NSUB):
                    nc.vector.bn_stats(
                        out=st[:, s, :],
                        in_=x_t[i][:, SUBLEN * s:SUBLEN * (s + 1)],
                    )
                mv = tp.tile([128, ADIM], F32, tag="bnmv", name=f"bnmv{i}")
                nc.vector.bn_aggr(out=mv, in_=st)
                me = sp.tile([128, 2], F32, tag=f"m1e2_{i}", name=f"m1e2_{i}")
                msq = tp.tile([128, 1], F32, tag="msq", name=f"msq{i}")
                nc.vector.tensor_mul(out=msq, in0=mv[:, 0:1], in1=mv[:, 0:1])
                nc.vector.tensor_copy(out=me[:, 0:1], in_=mv[:, 0:1])
                nc.vector.tensor_add(out=me[:, 1:2], in0=mv[:, 1:2], in1=msq)
                m1e2.append(me)

            psg = ps_pj.tile([8, 2], F32, tag="pj", name="psg")
            nc.tensor.matmul(psg, lhsT=indf_t[0], rhs=m1e2[0], start=True, stop=False)
            nc.tensor.matmul(psg, lhsT=indf_t[1], rhs=m1e2[1], start=False, stop=True)

            sg = sp.tile([8, 2], F32, tag="sg", name="sg")
            nc.scalar.mul(out=sg, in_=psg, mul=1.0 / 32.0)  # [mean_g, e2_g]
            vg = sp.tile([8, 1], F32, tag="vg", name="vg")
            nc.vector.tensor_mul(out=vg, in0=sg[:, 0:1], in1=sg[:, 0:1])
            nc.vector.tensor_sub(out=vg, in0=sg[:, 1:2], in1=vg)  # var_g
            nc.scalar.activation(out=vg, in_=vg, func=AF.Sqrt, bias=eps8)
            nc.vector.reciprocal(out=sg[:, 1:2], in_=vg)          # rstd_g

            h_t, scoff = [], []
            for i in range(2):
                psc = ps_pj.tile([128, 2], F32, tag="pj", name=f"psc{i}")
                nc.tensor.matmul(psc, lhsT=indb_t[i], rhs=sg, start=True, stop=True)
                sc = sp.tile([128, 1], F32, tag=f"sc{i}", name=f"sc{i}")
                off = sp.tile([128, 1], F32, tag=f"off{i}", name=f"off{i}")
                nc.vector.tensor_mul(out=sc, in0=psc[:, 1:2], in1=nw_t[i])
                nc.vector.tensor_mul(out=off, in0=psc[:, 0:1], in1=sc)
                nc.vector.tensor_sub(out=off, in0=nb_t[i], in1=off)
                ht = pp.tile([128, N], BF16, tag=f"h{i}", name=f"h{i}")
                h_t.append(ht)
                scoff.append((sc, off))

            # preload the exp table set while GroupNorm math is still in flight
            dummy = sp.tile([1, 1], BF16, tag="dummy", name="dummy")
            nc.scalar.activation(out=dummy, in_=eps8[0:1, 0:1], func=AF.Exp)

            # h = x*sc + off, chunks alternating ScalarE / VectorE
            for i in range(2):
                sc, off = scoff[i]
                for c4 in range(4):
                    csl = slice(1024 * c4, 1024 * (c4 + 1))
                    if (2 * i + c4) % 2 == 0:
                        nc.scalar.activation(
                            out=h_t[i][:, csl], in_=x_t[i][:, csl],
                            func=AF.Identity, bias=off, scale=sc)
                    else:
                        nc.vector.tensor_scalar(
                            out=h_t[i][:, csl], in0=x_t[i][:, csl],
                            scalar1=sc, scalar2=off, op0=ALU.mult, op1=ALU.add)

            # ================= Phase 2: qkv =================
            qT = pp.tile([128, N], BF16, tag="qT", name="qT")
            kT = pp.tile([128, N], BF16, tag="kT", name="kT")

            for n in range(NCH):
                nsl = slice(NCHUNK * n, NCHUNK * (n + 1))
                # k (no bias: cancels in softmax)
                psk = ps_sc.tile([128, NCHUNK], F32, tag="sc", name=f"k{n}")
                nc.tensor.matmul(psk, lhsT=wkT_t[0], rhs=h_t[0][:, nsl],
                                 start=True, stop=False)
                nc.tensor.matmul(psk, lhsT=wkT_t[1], rhs=h_t[1][:, nsl],
                                 start=False, stop=True)
                nc.vector.tensor_copy(out=kT[:, nsl], in_=psk)
                # q (+bias fused into the ScalarE copy)
                psq = ps_sc.tile([128, NCHUNK], F32, tag="sc", name=f"q{n}")
                nc.tensor.matmul(psq, lhsT=wqT_t[0], rhs=h_t[0][:, nsl],
                                 start=True, stop=False)
                nc.tensor.matmul(psq, lhsT=wqT_t[1], rhs=h_t[1][:, nsl],
                                 start=False, stop=True)
                nc.scalar.activation(out=qT[:, nsl], in_=psq,
                                     func=AF.Identity, bias=qb_t, scale=1.0)
                # v in [pixel, d] layout directly: out[pix, 130]
                for pc4 in range(4):
                    j = 4 * n + pc4
                    psl = slice(128 * j, 128 * (j + 1))
                    psv = ps_pj.tile([128, 130], F32, tag="pj", name=f"v{j}")
                    nc.tensor.matmul(psv, lhsT=h_t[0][:, psl], rhs=wvT_t[0],
                                     start=True, stop=False)
                    nc.tensor.matmul(psv, lhsT=h_t[1][:, psl], rhs=wvT_t[1],
                                     start=False, stop=True)
                    if j % 2 == 0:
                        nc.scalar.copy(out=v_all[:, j, :], in_=psv)
                    else:
                        nc.vector.tensor_copy(out=v_all[:, j, :], in_=psv)
                    nc.gpsimd.memset(v_all[:, j, 64:65], 1.0)
                    nc.gpsimd.memset(v_all[:, j, 129:130], 1.0)

            # ================= Phase 3: attention =================
            # per n: row-tiled concurrent score pairs, exp split ACT/DVE,
            # attnv skewed by SKEW, previous n's proj/copies interleaved.
            prev = None  # (po0, po1, n-1)

            def emit_tail_copies(po0, po1, pn):
                onrm = onp.tile([128, NCHUNK], BF16, tag="onrm", name=f"on{pn}")
                # denominators at partitions 0 (head A) and 64 (head B):
                # engine outputs must start at a 32-aligned partition
                sums = onp.tile([65, NCHUNK], F32, tag="sums", name=f"sm{pn}")
                nc.scalar.copy(out=onrm[0:64, :], in_=po0[0:64, :])
                nc.scalar.copy(out=onrm[64:128, :], in_=po1[0:64, :])
                nc.scalar.copy(out=sums[0:1, :], in_=po0[64:65, :])
                nc.scalar.copy(out=sums[64:65, :], in_=po1[64:65, :])
                return onrm, sums

            def emit_proj(onrm, ci, pn):
                pnsl = slice(NCHUNK * pn, NCHUNK * (pn + 1))
                csl = slice(128 * ci, 128 * (ci + 1))
                pjA = ps_pj.tile([128, NCHUNK], F32, tag="pj", name=f"pjA{pn}_{ci}")
                pjB = ps_pj.tile([128, NCHUNK], F32, tag="pj", name=f"pjB{pn}_{ci}")
                nc.tensor.matmul(pjA, lhsT=pwT_t[0:64, csl], rhs=onrm[0:64, :],
                                 start=True, stop=True)
                nc.tensor.matmul(pjB, lhsT=pwT_t[64:128, csl], rhs=onrm[64:128, :],
                                 start=True, stop=True)
                return pjA, pjB

            def emit_osb(pjA, pjB, ci, pn):
                pnsl = slice(NCHUNK * pn, NCHUNK * (pn + 1))
                csl = slice(128 * ci, 128 * (ci + 1))
                oA = obp.tile([128, NCHUNK], F32, tag="osb", name=f"oA{pn}_{ci}")
                oB = obp.tile([128, NCHUNK], F32, tag="osb", name=f"oB{pn}_{ci}")
                nc.scalar.copy(out=oA, in_=pjA)
                nc.scalar.copy(out=oB, in_=pjB)
                nc.sync.dma_start(out=outA_d[csl, pnsl], in_=oA)
                nc.sync.dma_start(out=outB_d[csl, pnsl], in_=oB)

            for n in range(NCH):
                nsl = slice(NCHUNK * n, NCHUNK * (n + 1))
                tail = None
                if prev is not None:
                    tail = emit_tail_copies(*prev)

                po0 = ps_po.tile([65, NCHUNK], F32, tag="po0", name=f"po0_{n}")
                po1 = ps_po.tile([65, NCHUNK], F32, tag="po1", name=f"po1_{n}")
                at0s, at1s = {}, {}
                pj_state = {}

                def emit_av(jj):
                    nc.tensor.matmul(
                        po0, lhsT=v_all[:, jj, 0:65], rhs=at0s.pop(jj),
                        start=(jj == 0), stop=(jj == MCH - 1))
                    nc.tensor.matmul(
                        po1, lhsT=v_all[:, jj, 65:130],
                        rhs=at1s.pop(jj).bitcast(BF16),
                        start=(jj == 0), stop=(jj == MCH - 1))

                for j in range(MCH):
                    jsl = slice(128 * j, 128 * (j + 1))
                    sA = ps_sc.tile([128, NCHUNK], F32, tag="sc", name=f"sA{n}_{j}")
                    sB = ps_sc.tile([128, NCHUNK], F32, tag="sc", name=f"sB{n}_{j}")
                    # adjacent K=64 matmuls on partition halves -> row-tiled,
                    # run concurrently on the PE
                    nc.tensor.matmul(sA, lhsT=kT[0:64, jsl], rhs=qT[0:64, nsl],
                                     start=True, stop=True)
                    nc.tensor.matmul(sB, lhsT=kT[64:128, jsl], rhs=qT[64:128, nsl],
                                     start=True, stop=True)
                    at0 = a0p.tile([128, NCHUNK], BF16, tag="a0", name=f"a0_{n}_{j}")
                    nc.scalar.activation(out=at0, in_=sA, func=AF.Exp, scale=SCALE)
                    at0s[j] = at0
                    at1 = a1p.tile([128, NCHUNK], I16, tag="a1", name=f"a1_{n}_{j}")
                    nc.vector.tensor_scalar(out=at1, in0=sB, scalar1=A_C,
                                            scalar2=B_C, op0=ALU.mult, op1=ALU.add)
                    at1s[j] = at1

                    if j >= SKEW:
                        emit_av(j - SKEW)

                    if tail is not None:
                        if j == 4:
                            pj_state[0] = emit_proj(tail[0], 0, n - 1)
                        elif j == 8:
                            emit_osb(*pj_state.pop(0), 0, n - 1)
                        elif j == 12:
                            pj_state[1] = emit_proj(tail[0], 1, n - 1)
                        elif j == 16:
                            emit_osb(*pj_state.pop(1), 1, n - 1)
                            pnsl = slice(NCHUNK * (n - 1), NCHUNK * n)
                            nc.sync.dma_start(out=sums_d[:, pnsl],
                                              in_=tail[1][0:65:64, :])

                for j in range(MCH - SKEW, MCH):
                    emit_av(j)
                prev = (po0, po1, n)

            # final tail (n = NCH-1)
            tail = emit_tail_copies(*prev)
            pjA, pjB = emit_proj(tail[0], 0, NCH - 1)
            emit_osb(pjA, pjB, 0, NCH - 1)
            pjA, pjB = emit_proj(tail[0], 1, NCH - 1)
            emit_osb(pjA, pjB, 1, NCH - 1)
            lsl = slice(NCHUNK * (NCH - 1), NCHUNK * NCH)
            nc.sync.dma_start(out=sums_d[:, lsl], in_=tail[1][0:65:64, :])

    nc.compile()
    return nc


_NC_CACHE = None


def _get_nc():
    global _NC_CACHE
    if _NC_CACHE is None:
        _NC_CACHE = build_bass()
    return _NC_CACHE


def _bf16(a):
    import ml_dtypes
    return np.ascontiguousarray(a).astype(ml_dtypes.bfloat16)


def _make_in_maps(x, norm_w, norm_b, qkv_w, qkv_b, proj_w):
    ch = np.arange(128)
    indf = np.zeros((2, 128, 8), np.float32)
    indb = np.zeros((2, 8, 128), np.float32)
    for i in range(2):
        g = (i * 128 + ch) // 32
        indf[i, ch, g] = 1.0
        indb[i, g, ch] = 1.0
    nw = norm_w.reshape(2, 128, 1).astype(np.float32)
    nb = norm_b.reshape(2, 128, 1).astype(np.float32)

    in_maps = []
    for core in range(8):
        b, hh = core // 2, core % 2
        sl = slice(128 * hh, 128 * (hh + 1))
        # wqT[ci][c, o] = qkv_w[128hh + o, 128ci + c]
        wq = qkv_w[sl]                      # [128, 256]
        wk = qkv_w[256 + 128 * hh:256 + 128 * (hh + 1)]
        wv = qkv_w[512 + 128 * hh:512 + 128 * (hh + 1)]  # [128, 256]
        wqT = np.stack([wq[:, 0:128].T, wq[:, 128:256].T], axis=0)
        wkT = np.stack([wk[:, 0:128].T, wk[:, 128:256].T], axis=0)
        wvT = np.zeros((2, 128, 130), np.float32)
        for ci in range(2):
            csl = slice(128 * ci, 128 * (ci + 1))
            wvT[ci, :, 0:64] = wv[0:64, csl].T     # head A
            wvT[ci, :, 65:129] = wv[64:128, csl].T  # head B
        qb = qkv_b[sl].reshape(128, 1).astype(np.float32)
        pwT = np.ascontiguousarray(proj_w[:, sl].T)  # [128, 256]
        in_maps.append({
            "x": np.ascontiguousarray(x[b].reshape(C, N)).astype(np.float32),
            "wqT": _bf16(wqT),
            "wkT": _bf16(wkT),
            "wvT": _bf16(wvT),
            "qb": qb,
            "pwT": _bf16(pwT),
            "nw": nw,
            "nb": nb,
            "indf": indf,
            "indb": indb,
        })
    return in_maps


def kernel(x, norm_w, norm_b, qkv_w, qkv_b, proj_w, proj_b, _trace=False, _tmpdir=None):
    x = np.asarray(x, np.float32)
    norm_w = np.asarray(norm_w, np.float32)
    norm_b = np.asarray(norm_b, np.float32)
    qkv_w = np.asarray(qkv_w, np.float32)
    qkv_b = np.asarray(qkv_b, np.float32)
    proj_w = np.asarray(proj_w, np.float32)
    proj_b = np.asarray(proj_b, np.float32)

    nc = _get_nc()
    in_maps = _make_in_maps(x, norm_w, norm_b, qkv_w, qkv_b, proj_w)
    kw = {}
    if _trace:
        kw = dict(trace=True, tmpdir=_tmpdir)
    res = run_bass_kernel_spmd(nc, in_maps, list(range(8)), **kw)

    # host: out = sum_heads partial/denominator + (proj_w @ v_bias + proj_b) + x
    vbias = qkv_b[512:768]
    const = (proj_w @ vbias + proj_b)[:, None].astype(np.float32)
    out = np.empty((B, C, H, W), np.float32)
    for b in range(B):
        acc = const + x[b].reshape(C, N)
        for hh in range(2):
            r = res.results[2 * b + hh]
            acc = acc + r["outA"] / r["sums"][0:1, :]
            acc = acc + r["outB"] / r["sums"][1:2, :]
        out[b] = acc.reshape(C, H, W)
    if _trace:
        return out, res
    return out


# revision 7
# speedup vs baseline: 1.3138x; 1.0265x over previous
"""AttentionBlock (GroupNorm + 4-head self-attention + proj + residual) on 8 TRN2 cores.

Sharding: core = 2*b + hh  (b = batch 0..3, hh = head-half 0..1).
Each core handles one batch image and 2 of the 4 heads.

Engine-level structure:
 - The two heads' score matmuls (K=64 contraction each) are issued adjacently so
   the PE runs them concurrently via row tiling (partitions 0:64 / 64:128), into
   one [128, 2, 512] two-bank PSUM tile.
 - The softmax exp (N^2 logits/head, the elementwise bottleneck) alternates by
   k-chunk between ScalarE (exact Exp LUT) and VectorE (Schraudolph approximate
   exp: int16(S*a+b) whose bits, read as bf16, equal C*2^(S*scale*log2e); the
   constant C cancels in softmax).  One 1024-wide instruction covers both heads.
 - k bias is dropped (constant per query -> cancels in softmax); q bias is fused
   into the ScalarE PSUM->SBUF copy; v bias is folded into the output on host.
 - flash-style combine on host: the device ships raw per-head proj partials
   [256, N] and per-head softmax denominators [2, N]; the host computes
   out = sum_heads partial/denominator + const + residual.
 - x is loaded as bf16 (device only needs it for GroupNorm/qkv; the residual is
   added on host in fp32).
"""

import sys

sys.path.insert(0, "/opt/trn_rl_repo")

import numpy as np  # noqa: E402

import concourse.bacc as bacc  # noqa: E402
import concourse.tile as tile  # noqa: E402
from concourse import mybir  # noqa: E402
from concourse.bass_utils import run_bass_kernel_spmd  # noqa: E402

F32 = mybir.dt.float32
BF16 = mybir.dt.bfloat16
I16 = mybir.dt.int16
AF = mybir.ActivationFunctionType
ALU = mybir.AluOpType

# Problem constants (hardcoded per contract)
B, C, H, W = 4, 256, 64, 64
N = H * W          # 4096 pixels
NH, HD = 4, 64     # heads, head dim
GROUPS = 8
EPS = 1e-5
SCALE = HD ** -0.5  # 0.125

NCHUNK = 512            # pixel chunk (matmul moving dim)
NCH = N // NCHUNK       # 8
MCH = N // 128          # 32 k-chunks of 128 pixels
SKEW = 3                # attnv lags scores by SKEW k-chunks

# Schraudolph exp-as-bf16-bits constants (DVE rounds to nearest; verified on HW)
LOG2E = 1.4426950408889634
A_C = SCALE * LOG2E * 128.0        # 23.0831...
# sigma centers the mean multiplicative ratio at 1.0 (the approx chunks mix
# with exact-exp chunks inside one softmax, so the constant must not bias)
B_C = 128.0 * (127.0 - 0.05641)    # 16248.78


def build_bass():
    nc = bacc.Bacc("TRN2", target_bir_lowering=False, debug=False)

    # ---- DRAM I/O (per-core shards fed via in_maps) ----
    xd = nc.dram_tensor("x", [C, N], BF16, kind="ExternalInput")
    wqT_d = nc.dram_tensor("wqT", [2, 128, 128], BF16, kind="ExternalInput")
    wkT_d = nc.dram_tensor("wkT", [2, 128, 128], BF16, kind="ExternalInput")
    wvT_d = nc.dram_tensor("wvT", [2, 128, 130], BF16, kind="ExternalInput")
    qb_d = nc.dram_tensor("qb", [128, 1], F32, kind="ExternalInput")
    pwT_d = nc.dram_tensor("pwT", [128, 256], BF16, kind="ExternalInput")
    nw_d = nc.dram_tensor("nw", [2, 128, 1], F32, kind="ExternalInput")
    nb_d = nc.dram_tensor("nb", [2, 128, 1], F32, kind="ExternalInput")
    indf_d = nc.dram_tensor("indf", [2, 128, 8], F32, kind="ExternalInput")
    indb_d = nc.dram_tensor("indb", [2, 8, 128], F32, kind="ExternalInput")
    outA_d = nc.dram_tensor("outA", [C, N], F32, kind="ExternalOutput")
    outB_d = nc.dram_tensor("outB", [C, N], F32, kind="ExternalOutput")
    sums_d = nc.dram_tensor("sums", [2, N], F32, kind="ExternalOutput")

    with tile.TileContext(nc) as tc:
        with (
            tc.tile_pool(name="persist", bufs=1) as pp,
            tc.tile_pool(name="tmp", bufs=4) as tp,
            tc.tile_pool(name="small", bufs=4) as sp,
            tc.tile_pool(name="a0pool", bufs=3) as a0p,
            tc.tile_pool(name="a1pool", bufs=3) as a1p,
            tc.tile_pool(name="onpool", bufs=2) as onp,
            tc.tile_pool(name="osbpool", bufs=4) as obp,
            tc.tile_pool(name="ps_sc", bufs=2, space="PSUM") as ps_sc,
            tc.tile_pool(name="ps_po", bufs=1, space="PSUM") as ps_po,
            tc.tile_pool(name="ps_pj", bufs=2, space="PSUM") as ps_pj,
        ):
            # ================= Phase 0: loads & constants =================
            # x as bf16, split across the two DMA queues (SP + Activation)
            x_t = []
            for i in range(2):
                xt = pp.tile([128, N], BF16, tag=f"x{i}", name=f"x{i}")
                for c4 in range(4):
                    eng = nc.sync if c4 % 2 == 0 else nc.scalar
                    eng.dma_start(
                        out=xt[:, 1024 * c4:1024 * (c4 + 1)],
                        in_=xd[128 * i:128 * (i + 1), 1024 * c4:1024 * (c4 + 1)])
                x_t.append(xt)

            wqT_t, wkT_t, wvT_t = [], [], []
            for ci in range(2):
                t = pp.tile([128, 128], BF16, tag=f"wq{ci}", name=f"wq{ci}")
                nc.sync.dma_start(out=t, in_=wqT_d[ci])
                wqT_t.append(t)
                t = pp.tile([128, 128], BF16, tag=f"wk{ci}", name=f"wk{ci}")
                nc.sync.dma_start(out=t, in_=wkT_d[ci])
                wkT_t.append(t)
                t = pp.tile([128, 130], BF16, tag=f"wv{ci}", name=f"wv{ci}")
                nc.sync.dma_start(out=t, in_=wvT_d[ci])
                wvT_t.append(t)

            qb_t = sp.tile([128, 1], F32, tag="qb", name="qb")
            nc.sync.dma_start(out=qb_t, in_=qb_d[:, :])
            pwT_t = pp.tile([128, 256], BF16, tag="pw", name="pw")
            nc.sync.dma_start(out=pwT_t, in_=pwT_d[:, :])

            nw_t, nb_t, indf_t, indb_t = [], [], [], []
            for i in range(2):
                t1 = sp.tile([128, 1], F32, tag=f"nw{i}", name=f"nw{i}")
                nc.sync.dma_start(out=t1, in_=nw_d[i])
                nw_t.append(t1)
                t2 = sp.tile([128, 1], F32, tag=f"nb{i}", name=f"nb{i}")
                nc.sync.dma_start(out=t2, in_=nb_d[i])
                nb_t.append(t2)
                t3 = sp.tile([128, 8], F32, tag=f"indf{i}", name=f"indf{i}")
                nc.sync.dma_start(out=t3, in_=indf_d[i])
                indf_t.append(t3)
                t4 = sp.tile([8, 128], F32, tag=f"indb{i}", name=f"indb{i}")
                nc.sync.dma_start(out=t4, in_=indb_d[i])
                indb_t.append(t4)

            eps8 = sp.tile([8, 1], F32, tag="eps8", name="eps8")
            nc.vector.memset(eps8, EPS)

            # v_all[:, j, :] = [vA(64) | onesA(1) | vB(64) | onesB(1)]
            v_all = pp.tile([128, MCH, 130], BF16, tag="v_all", name="v_all")

            # ================= Phase 1: GroupNorm =================
            SDIM = nc.vector.BN_STATS_DIM   # 6
            ADIM = nc.vector.BN_AGGR_DIM    # 2
            NSUB = N // nc.vector.BN_STATS_FMAX if N > nc.vector.BN_STATS_FMAX else 1
            SUBLEN = N // NSUB

            m1e2 = []
            for i in range(2):
                st = tp.tile([128, NSUB, SDIM], F32, tag="bnst", name=f"bnst{i}")
                for s in range(NSUB):
                    nc.vector.bn_stats(
                        out=st[:, s, :],
                        in_=x_t[i][:, SUBLEN * s:SUBLEN * (s + 1)],
                    )
                mv = tp.tile([128, ADIM], F32, tag="bnmv", name=f"bnmv{i}")
                nc.vector.bn_aggr(out=mv, in_=st)
                me = sp.tile([128, 2], F32, tag=f"m1e2_{i}", name=f"m1e2_{i}")
                msq = tp.tile([128, 1], F32, tag="msq", name=f"msq{i}")
                nc.vector.tensor_mul(out=msq, in0=mv[:, 0:1], in1=mv[:, 0:1])
                nc.vector.tensor_copy(out=me[:, 0:1], in_=mv[:, 0:1])
                nc.vector.tensor_add(out=me[:, 1:2], in0=mv[:, 1:2], in1=msq)
                m1e2.append(me)

            psg = ps_pj.tile([8, 2], F32, tag="pj", name="psg")
            nc.tensor.matmul(psg, lhsT=indf_t[0], rhs=m1e2[0], start=True, stop=False)
            nc.tensor.matmul(psg, lhsT=indf_t[1], rhs=m1e2[1], start=False, stop=True)

            sg = sp.tile([8, 2], F32, tag="sg", name="sg")
            nc.scalar.mul(out=sg, in_=psg, mul=1.0 / 32.0)  # [mean_g, e2_g]
            vg = sp.tile([8, 1], F32, tag="vg", name="vg")
            nc.vector.tensor_mul(out=vg, in0=sg[:, 0:1], in1=sg[:, 0:1])
            nc.vector.tensor_sub(out=vg, in0=sg[:, 1:2], in1=vg)  # var_g
            nc.scalar.activation(out=vg, in_=vg, func=AF.Sqrt, bias=eps8)
            nc.vector.reciprocal(out=sg[:, 1:2], in_=vg)          # rstd_g

            h_t, scoff = [], []
            for i in range(2):
                psc = ps_pj.tile([128, 2], F32, tag="pj", name=f"psc{i}")
                nc.tensor.matmul(psc, lhsT=indb_t[i], rhs=sg, start=True, stop=True)
                sc = sp.tile([128, 1], F32, tag=f"sc{i}", name=f"sc{i}")
                off = sp.tile([128, 1], F32, tag=f"off{i}", name=f"off{i}")
                nc.vector.tensor_mul(out=sc, in0=psc[:, 1:2], in1=nw_t[i])
                nc.vector.tensor_mul(out=off, in0=psc[:, 0:1], in1=sc)
                nc.vector.tensor_sub(out=off, in0=nb_t[i], in1=off)
                ht = pp.tile([128, N], BF16, tag=f"h{i}", name=f"h{i}")
                h_t.append(ht)
                scoff.append((sc, off))

            # preload the exp table set while GroupNorm math is still in flight
            dummy = sp.tile([1, 1], BF16, tag="dummy", name="dummy")
            nc.scalar.activation(out=dummy, in_=eps8[0:1, 0:1], func=AF.Exp)

            # h = x*sc + off, chunks alternating ScalarE / VectorE
            for i in range(2):
                sc, off = scoff[i]
                for c4 in range(4):
                    csl = slice(1024 * c4, 1024 * (c4 + 1))
                    if (2 * i + c4) % 2 == 0:
                        nc.scalar.activation(
                            out=h_t[i][:, csl], in_=x_t[i][:, csl],
                            func=AF.Identity, bias=off, scale=sc)
                    else:
                        nc.vector.tensor_scalar(
                            out=h_t[i][:, csl], in0=x_t[i][:, csl],
                            scalar1=sc, scalar2=off, op0=ALU.mult, op1=ALU.add)

            # ================= Phase 2: k, v (q deferred) =================
            qT = pp.tile([128, N], BF16, tag="qT", name="qT")
            kT = pp.tile([128, N], BF16, tag="kT", name="kT")

            def emit_q(n):
                nsl = slice(NCHUNK * n, NCHUNK * (n + 1))
                psq = ps_sc.tile([128, 2, NCHUNK], F32, tag="sc", name=f"q{n}")
                nc.tensor.matmul(psq[:, 0, :], lhsT=wqT_t[0], rhs=h_t[0][:, nsl],
                                 start=True, stop=False)
                nc.tensor.matmul(psq[:, 0, :], lhsT=wqT_t[1], rhs=h_t[1][:, nsl],
                                 start=False, stop=True)
                nc.scalar.activation(out=qT[:, nsl], in_=psq[:, 0, :],
                                     func=AF.Identity, bias=qb_t, scale=1.0)

            for n in range(NCH):
                nsl = slice(NCHUNK * n, NCHUNK * (n + 1))
                psk = ps_sc.tile([128, 2, NCHUNK], F32, tag="sc", name=f"k{n}")
                nc.tensor.matmul(psk[:, 0, :], lhsT=wkT_t[0], rhs=h_t[0][:, nsl],
                                 start=True, stop=False)
                nc.tensor.matmul(psk[:, 0, :], lhsT=wkT_t[1], rhs=h_t[1][:, nsl],
                                 start=False, stop=True)
                nc.vector.tensor_copy(out=kT[:, nsl], in_=psk[:, 0, :])
                # v in [pixel, d] layout directly: out[pix, 130]
                for pc4 in range(4):
                    j = 4 * n + pc4
                    psl = slice(128 * j, 128 * (j + 1))
                    psv = ps_pj.tile([128, 130], F32, tag="pj", name=f"v{j}")
                    nc.tensor.matmul(psv, lhsT=h_t[0][:, psl], rhs=wvT_t[0],
                                     start=True, stop=False)
                    nc.tensor.matmul(psv, lhsT=h_t[1][:, psl], rhs=wvT_t[1],
                                     start=False, stop=True)
                    if j % 2 == 0:
                        nc.scalar.copy(out=v_all[:, j, :], in_=psv)
                    else:
                        nc.vector.tensor_copy(out=v_all[:, j, :], in_=psv)
                    nc.gpsimd.memset(v_all[:, j, 64:65], 1.0)
                    nc.gpsimd.memset(v_all[:, j, 129:130], 1.0)

            emit_q(0)
            emit_q(1)

            # ================= Phase 3: attention =================
            prev = None  # (po0, po1, n-1)

            def emit_tail_copies(po0, po1, pn):
                onrm = onp.tile([128, NCHUNK], BF16, tag="onrm", name=f"on{pn}")
                # denominators at partitions 0 (head A) and 64 (head B)
                sums = onp.tile([65, NCHUNK], F32, tag="sums", name=f"sm{pn}")
                nc.scalar.copy(out=onrm[0:64, :], in_=po0[0:64, :])
                nc.scalar.copy(out=sums[0:1, :], in_=po0[64:65, :])
                nc.vector.tensor_copy(out=onrm[64:128, :], in_=po1[0:64, :])
                nc.vector.tensor_copy(out=sums[64:65, :], in_=po1[64:65, :])
                return onrm, sums

            def emit_proj(onrm, ci, pn):
                csl = slice(128 * ci, 128 * (ci + 1))
                pjA = ps_pj.tile([128, NCHUNK], F32, tag="pj", name=f"pjA{pn}_{ci}")
                pjB = ps_pj.tile([128, NCHUNK], F32, tag="pj", name=f"pjB{pn}_{ci}")
                nc.tensor.matmul(pjA, lhsT=pwT_t[0:64, csl], rhs=onrm[0:64, :],
                                 start=True, stop=True)
                nc.tensor.matmul(pjB, lhsT=pwT_t[64:128, csl], rhs=onrm[64:128, :],
                                 start=True, stop=True)
                return pjA, pjB

            def emit_osb(pjA, pjB, ci, pn):
                pnsl = slice(NCHUNK * pn, NCHUNK * (pn + 1))
                csl = slice(128 * ci, 128 * (ci + 1))
                oA = obp.tile([128, NCHUNK], F32, tag="osb", name=f"oA{pn}_{ci}")
                oB = obp.tile([128, NCHUNK], F32, tag="osb", name=f"oB{pn}_{ci}")
                nc.scalar.copy(out=oA, in_=pjA)
                nc.vector.tensor_copy(out=oB, in_=pjB)
                nc.sync.dma_start(out=outA_d[csl, pnsl], in_=oA)
                nc.sync.dma_start(out=outB_d[csl, pnsl], in_=oB)

            for n in range(NCH):
                nsl = slice(NCHUNK * n, NCHUNK * (n + 1))
                tail = None
                if prev is not None:
                    tail = emit_tail_copies(*prev)

                po0 = ps_po.tile([65, NCHUNK], F32, tag="po0", name=f"po0_{n}")
                po1 = ps_po.tile([65, NCHUNK], F32, tag="po1", name=f"po1_{n}")
                ats = {}

                def emit_av(jj):
                    at = ats.pop(jj)
                    r0, r1 = at[:, 0, :], at[:, 1, :]
                    if jj % 2 == 1:
                        r0, r1 = r0.bitcast(BF16), r1.bitcast(BF16)
                    nc.tensor.matmul(po0, lhsT=v_all[:, jj, 0:65], rhs=r0,
                                     start=(jj == 0), stop=(jj == MCH - 1))
                    nc.tensor.matmul(po1, lhsT=v_all[:, jj, 65:130], rhs=r1,
                                     start=(jj == 0), stop=(jj == MCH - 1))

                for j in range(MCH):
                    jsl = slice(128 * j, 128 * (j + 1))
                    sAB = ps_sc.tile([128, 2, NCHUNK], F32, tag="sc",
                                     name=f"s{n}_{j}")
                    # adjacent K=64 matmuls on partition halves -> row-tiled,
                    # run concurrently on the PE
                    nc.tensor.matmul(sAB[:, 0, :], lhsT=kT[0:64, jsl],
                                     rhs=qT[0:64, nsl], start=True, stop=True)
                    nc.tensor.matmul(sAB[:, 1, :], lhsT=kT[64:128, jsl],
                                     rhs=qT[64:128, nsl], start=True, stop=True)
                    # one 1024-wide exp covering both heads, alternating engine
                    if j % 2 == 0:
                        at = a0p.tile([128, 2, NCHUNK], BF16, tag="a0",
                                      name=f"a_{n}_{j}")
                        nc.scalar.activation(out=at, in_=sAB, func=AF.Exp,
                                             scale=SCALE)
                    else:
                        at = a1p.tile([128, 2, NCHUNK], I16, tag="a1",
                                      name=f"a_{n}_{j}")
                        nc.vector.tensor_scalar(out=at, in0=sAB, scalar1=A_C,
                                                scalar2=B_C, op0=ALU.mult,
                                                op1=ALU.add)
                    ats[j] = at

                    if j >= SKEW:
                        emit_av(j - SKEW)

                    if tail is not None:
                        if j == 4:
                            pj_state = emit_proj(tail[0], 0, n - 1)
                        elif j == 8:
                            emit_osb(*pj_state, 0, n - 1)
                        elif j == 12:
                            pj_state = emit_proj(tail[0], 1, n - 1)
                        elif j == 16:
                            emit_osb(*pj_state, 1, n - 1)
                            pnsl = slice(NCHUNK * (n - 1), NCHUNK * n)
                            nc.sync.dma_start(out=sums_d[:, pnsl],
                                              in_=tail[1][0:65:64, :])
                    if j == 20 and n + 2 < NCH:
                        emit_q(n + 2)

                for j in range(MCH - SKEW, MCH):
                    emit_av(j)
                prev = (po0, po1, n)

            # final tail (n = NCH-1)
            tail = emit_tail_copies(*prev)
            pjA, pjB = emit_proj(tail[0], 0, NCH - 1)
            emit_osb(pjA, pjB, 0, NCH - 1)
            pjA, pjB = emit_proj(tail[0], 1, NCH - 1)
            emit_osb(pjA, pjB, 1, NCH - 1)
            lsl = slice(NCHUNK * (NCH - 1), NCHUNK * NCH)
            nc.sync.dma_start(out=sums_d[:, lsl], in_=tail[1][0:65:64, :])

    nc.compile()
    return nc


_NC_CACHE = None


def _get_nc():
    global _NC_CACHE
    if _NC_CACHE is None:
        _NC_CACHE = build_bass()
    return _NC_CACHE


def _bf16(a):
    import ml_dtypes
    return np.ascontiguousarray(a).astype(ml_dtypes.bfloat16)


def _make_in_maps(x, norm_w, norm_b, qkv_w, qkv_b, proj_w):
    ch = np.arange(128)
    indf = np.zeros((2, 128, 8), np.float32)
    indb = np.zeros((2, 8, 128), np.float32)
    for i in range(2):
        g = (i * 128 + ch) // 32
        indf[i, ch, g] = 1.0
        indb[i, g, ch] = 1.0
    nw = norm_w.reshape(2, 128, 1).astype(np.float32)
    nb = norm_b.reshape(2, 128, 1).astype(np.float32)

    in_maps = []
    for core in range(8):
        b, hh = core // 2, core % 2
        sl = slice(128 * hh, 128 * (hh + 1))
        wq = qkv_w[sl]                      # [128, 256]
        wk = qkv_w[256 + 128 * hh:256 + 128 * (hh + 1)]
        wv = qkv_w[512 + 128 * hh:512 + 128 * (hh + 1)]
        wqT = np.stack([wq[:, 0:128].T, wq[:, 128:256].T], axis=0)
        wkT = np.stack([wk[:, 0:128].T, wk[:, 128:256].T], axis=0)
        wvT = np.zeros((2, 128, 130), np.float32)
        for ci in range(2):
            csl = slice(128 * ci, 128 * (ci + 1))
            wvT[ci, :, 0:64] = wv[0:64, csl].T      # head A
            wvT[ci, :, 65:129] = wv[64:128, csl].T  # head B
        qb = qkv_b[sl].reshape(128, 1).astype(np.float32)
        pwT = np.ascontiguousarray(proj_w[:, sl].T)  # [128, 256]
        in_maps.append({
            "x": _bf16(x[b].reshape(C, N)),
            "wqT": _bf16(wqT),
            "wkT": _bf16(wkT),
            "wvT": _bf16(wvT),
            "qb": qb,
            "pwT": _bf16(pwT),
            "nw": nw,
            "nb": nb,
            "indf": indf,
            "indb": indb,
        })
    return in_maps


def kernel(x, norm_w, norm_b, qkv_w, qkv_b, proj_w, proj_b, _trace=False, _tmpdir=None):
    x = np.asarray(x, np.float32)
    norm_w = np.asarray(norm_w, np.float32)
    norm_b = np.asarray(norm_b, np.float32)
    qkv_w = np.asarray(qkv_w, np.float32)
    qkv_b = np.asarray(qkv_b, np.float32)
    proj_w = np.asarray(proj_w, np.float32)
    proj_b = np.asarray(proj_b, np.float32)

    nc = _get_nc()
    in_maps = _make_in_maps(x, norm_w, norm_b, qkv_w, qkv_b, proj_w)
    kw = {}
    if _trace:
        kw = dict(trace=True, tmpdir=_tmpdir)
    res = run_bass_kernel_spmd(nc, in_maps, list(range(8)), **kw)

    # host: flash-style combine of per-head partials + residual
    vbias = qkv_b[512:768]
    const = (proj_w @ vbias + proj_b)[:, None].astype(np.float32)
    out = np.empty((B, C, H, W), np.float32)
    for b in range(B):
        acc = const + x[b].reshape(C, N)
        for hh in range(2):
            r = res.results[2 * b + hh]
            acc = acc + r["outA"] / r["sums"][0:1, :]
            acc = acc + r["outB"] / r["sums"][1:2, :]
        out[b] = acc.reshape(C, H, W)
    if _trace:
        return out, res
    return out
